# revision 14
# baseline (speedup 1.0000x reference)
"""GCMC layer kernel for 8 Trainium2 NeuronCores (Bass/Tile).

Strategy (dest-sharded message passing, one shared SPMD NEFF):
  Phase 1: each core transforms its node shard: xu[row, 5*128] = featT.T @ Wcat
           (fp32r matmuls), scaled by cj, cast to bf16 -> local DRAM table.
  AllGather: full bf16 message tables on every core.
  Phase 2: per (dest-tile, rating): dma_gather per-edge source messages
           (int16 indices rebased per 32768-row window), build one-hot S from
           dest columns on DVE, accumulate h_T[k, dest] = M.T @ S in PSUM.
  Phase 3: FC h_T.T @ fc_w accumulated over ratings (fp32r), scale by ci,
           add bias, write output shards; host concatenates.

Host precomputes: transposed features, per-core edge orderings sorted by
(band, rating, window, dest-tile), a max-over-cores common chunk schedule so
all 8 cores share one instruction stream, int16 gather indices, and per-edge
dest-column tables.
"""
import numpy as np
import ml_dtypes

from concourse import bacc, mybir, tile
from concourse.bass_utils import run_bass_kernel_spmd

# Problem constants (from spec)
R = 5
NU = 100000
NI = 50000
DIN = 512
K = 128
MSG = R * K            # 640
OUT = 256
E = 200000
NCORES = 8

U_SH = NU // NCORES    # 12500 users per core
I_SH = NI // NCORES    # 6250 movies per core
U_PAD = 12544          # table rows per user shard (98 * 128)
I_PAD = 6272           # table rows per movie shard (49 * 128)
U_ROWS = U_PAD * NCORES   # 100352 (4 windows of 25088)
I_ROWS = I_PAD * NCORES   # 50176  (2 windows of 25088)
WIN = 25088               # equal-size index windows (<= 32768 for int16)

T_DST = 96             # dest-tile span for phase 2/3
U_TILES = -(-U_PAD // T_DST)   # 131 user dest tiles
I_TILES = -(-I_PAD // T_DST)   # 66 movie dest tiles
U_OPAD = U_TILES * T_DST       # 12576 output rows per core
I_OPAD = I_TILES * T_DST       # 6336
BAND = 16              # dest tiles per gather band

BF16 = ml_dtypes.bfloat16

_CACHE = {}


def _shard_pad_T(x, sh, pad):
    """[N, D] -> per-core transposed padded [D, pad] fp32 list."""
    out = []
    for c in range(NCORES):
        blk = x[c * sh:(c + 1) * sh]
        t = np.zeros((x.shape[1], pad), np.float32)
        t[:, :sh] = blk.T
        out.append(np.ascontiguousarray(t))
    return out


def _shard_pad_vec(v, sh, pad):
    out = []
    for c in range(NCORES):
        t = np.zeros((pad, 1), np.float32)
        t[:sh, 0] = v[c * sh:(c + 1) * sh, 0]
        out.append(t)
    return out


def _prep_direction(edge_src, edge_dst, dst_sh, n_tiles, src_sh, src_pad, n_win):
    """Shared chunk schedule + per-core gather-index / dest-column arrays.

    Slot layout: bands outer; within band: r, then w, then t. All 8 cores pad
    each (r, t, w) cell to the max-over-cores chunk count so the instruction
    stream is identical.

    Returns dict:
      calls[b][r]: list of (w, nch, slot_off, col_off) gather calls
      slab_cols[b][r]: total chunk columns of slab (b, r)
      chunk_pos[r][t]: list of (col_in_slab, gcol) per chunk
      idx16[c], dcols[c]: per-core device input arrays
    """
    core_of = edge_dst // dst_sh
    dloc = edge_dst - core_of * dst_sh
    t_of = dloc // T_DST
    dcol_of = dloc - t_of * T_DST
    srow = (edge_src // src_sh) * src_pad + (edge_src % src_sh)
    w_of = srow // WIN

    n_bands = -(-n_tiles // BAND)
    counts = np.zeros((NCORES, R, n_tiles, n_win), np.int64)
    for r in range(R):
        for c in range(NCORES):
            m = core_of[r] == c
            key = t_of[r][m] * n_win + w_of[r][m]
            counts[c, r] += np.bincount(key, minlength=n_tiles * n_win).reshape(
                n_tiles, n_win)
    cell_chunks = -(-counts.max(axis=0) // 128)       # [R, n_tiles, n_win]

    calls = [[[] for _ in range(R)] for _ in range(n_bands)]
    slab_cols = [[0] * R for _ in range(n_bands)]
    chunk_pos = [[[] for _ in range(n_tiles)] for _ in range(R)]
    cell_slot = np.zeros((R, n_tiles, n_win), np.int64)
    slot_off = 0
    gcol = 0
    for b in range(n_bands):
        t0, t1 = b * BAND, min((b + 1) * BAND, n_tiles)
        for r in range(R):
            col = 0                                   # column within slab (b, r)
            for w in range(n_win):
                nch = int(cell_chunks[r, t0:t1, w].sum())
                if nch == 0:
                    continue
                calls[b][r].append((w, nch, slot_off, col))
                cc_cum = col
                for t in range(t0, t1):
                    cc = int(cell_chunks[r, t, w])
                    cell_slot[r, t, w] = slot_off + (cc_cum - col) * 128
                    for j in range(cc):
                        chunk_pos[r][t].append((cc_cum + j, gcol + cc_cum - col + j))
                    cc_cum += cc
                slot_off += nch * 128
                gcol += nch
                col = cc_cum
            slab_cols[b][r] = col
    total_slots = slot_off
    total_chunks = gcol

    # per (r, t): order chunks by (w, j) — matches append order (w outer) ✓
    idx16, dcols = [], []
    for c in range(NCORES):
        idx_arr = np.zeros(total_slots, np.int16)
        dcol_arr = np.full(total_slots, -1, np.int16)
        for r in range(R):
            m = core_of[r] == c
            tt, ww = t_of[r][m], w_of[r][m]
            rows, dc = srow[r][m], dcol_of[r][m]
            order = np.lexsort((ww, tt))
            tt, ww, rows, dc = tt[order], ww[order], rows[order], dc[order]
            key = tt * n_win + ww
            uniq, first = np.unique(key, return_index=True)
            cnts = np.diff(np.append(first, len(key)))
            rank = np.arange(len(key)) - np.repeat(first, cnts)
            pos = cell_slot[r].reshape(-1)[key] + rank
            idx_arr[pos] = (rows - ww * WIN).astype(np.int16)
            dcol_arr[pos] = dc.astype(np.int16)
        iw = np.tile(idx_arr.reshape(-1, 16).T, (8, 1)).copy()
        dw = np.ascontiguousarray(dcol_arr.reshape(-1, 128).T)
        idx16.append(iw)
        dcols.append(dw)

    return dict(calls=calls, slab_cols=slab_cols, chunk_pos=chunk_pos,
                idx16=idx16, dcols=dcols, total_slots=total_slots,
                total_chunks=total_chunks, n_bands=n_bands, n_win=n_win,
                n_tiles=n_tiles,
                max_slab_cols=max(max(sc) for sc in slab_cols))


def _build_program(ui, iu, verbose=False):
    import os
    lim = os.environ.get("PHASE_LIMIT", "all")
    nc = bacc.Bacc("TRN2", target_bir_lowering=False, debug=False,
                   num_devices=NCORES)
    f32, f32r, bf16, i16 = (mybir.dt.float32, mybir.dt.float32r,
                            mybir.dt.bfloat16, mybir.dt.int16)

    def inp(name, shape, dt):
        return nc.dram_tensor(name, shape, dt, kind="ExternalInput").ap()

    featT_u = inp("featT_u", [DIN, U_PAD], f32)
    featT_i = inp("featT_i", [DIN, I_PAD], f32)
    wcat_u = inp("wcat_u", [DIN, MSG], f32)
    wcat_i = inp("wcat_i", [DIN, MSG], f32)
    cj_u = inp("cj_u", [U_PAD, 1], f32)
    cj_i = inp("cj_i", [I_PAD, 1], f32)
    ci_u = inp("ci_u", [U_OPAD, 1], f32)
    ci_i = inp("ci_i", [I_OPAD, 1], f32)
    fcw_u = inp("fcw_u", [R, K, OUT], f32)
    fcw_i = inp("fcw_i", [R, K, OUT], f32)
    bias_u = inp("bias_u", [128, OUT], f32)
    bias_i = inp("bias_i", [128, OUT], f32)
    iota_t = inp("iota_t", [128, T_DST], bf16)
    idx_ui = inp("idx_ui", [128, ui["total_slots"] // 16], i16)
    idx_iu = inp("idx_iu", [128, iu["total_slots"] // 16], i16)
    dcol_ui = inp("dcol_ui", [128, ui["total_chunks"]], i16)
    dcol_iu = inp("dcol_iu", [128, iu["total_chunks"]], i16)

    out_u = nc.dram_tensor("out_u", [U_OPAD, OUT], f32, kind="ExternalOutput").ap()
    out_i = nc.dram_tensor("out_i", [I_OPAD, OUT], f32, kind="ExternalOutput").ap()
    dbg = nc.dram_tensor("dbg", [256, MSG], bf16, kind="ExternalOutput").ap()

    ag_in_u = nc.dram_tensor("ag_in_u", [U_PAD, MSG], bf16).ap()
    ag_in_i = nc.dram_tensor("ag_in_i", [I_PAD, MSG], bf16).ap()
    xu_all = nc.dram_tensor("xu_all", [U_ROWS, MSG], bf16, addr_space="Shared").ap()
    xi_all = nc.dram_tensor("xi_all", [I_ROWS, MSG], bf16, addr_space="Shared").ap()

    with tile.TileContext(nc) as tc:
        with tc.tile_pool(name="const", bufs=1) as cpool:
            # ---------- constants (staging pool closes after copies) ----------
            w_sb, fc_sb, bias_sb, dcol_sb = {}, {}, {}, {}
            with tc.tile_pool(name="stage", bufs=2) as stg:
                for nm, wd in (("u", wcat_u), ("i", wcat_i)):
                    wt = stg.tile([128, 4, MSG], f32, tag="wst")
                    nc.sync.dma_start(out=wt[:], in_=wd.rearrange(
                        "(c p) m -> p c m", p=128))
                    wr = cpool.tile([128, 4, MSG], f32r, tag=f"wr_{nm}")
                    nc.vector.tensor_copy(out=wr[:], in_=wt[:])
                    w_sb[nm] = wr
                for nm, fd in (("u", fcw_u), ("i", fcw_i)):
                    ft = stg.tile([128, R, OUT], f32, tag="fst")
                    nc.sync.dma_start(out=ft[:], in_=fd.rearrange("r p o -> p r o"))
                    fr = cpool.tile([128, R, OUT], f32r, tag=f"fcr_{nm}")
                    nc.vector.tensor_copy(out=fr[:], in_=ft[:])
                    fc_sb[nm] = fr
                for nm, dd, tot in (("ui", dcol_ui, ui["total_chunks"]),
                                    ("iu", dcol_iu, iu["total_chunks"])):
                    dt_ = stg.tile([128, tot], i16, tag="dst")
                    nc.sync.dma_start(out=dt_[:], in_=dd[:])
                    db = cpool.tile([128, tot], f32, tag=f"dcb_{nm}")
                    nc.vector.tensor_copy(out=db[:], in_=dt_[:])
                    dcol_sb[nm] = db
            for nm, bd in (("u", bias_u), ("i", bias_i)):
                bt = cpool.tile([128, OUT], f32, tag=f"b_{nm}")
                nc.sync.dma_start(out=bt[:], in_=bd[:])
                bias_sb[nm] = bt
            iota_sb = cpool.tile([128, T_DST], bf16)
            nc.sync.dma_start(out=iota_sb[:], in_=iota_t[:])

            # ---------- phase 1: transform own shards ----------
            with (
                tc.tile_pool(name="p1sb", bufs=3) as p1sb,
                tc.tile_pool(name="p1ps", bufs=3, space="PSUM") as p1ps,
            ):
                for nm, featT, cjd, agin, npad in (
                    ("u", featT_u, cj_u, ag_in_u, U_PAD),
                    ("i", featT_i, cj_i, ag_in_i, I_PAD),
                ):
                    for t in range(npad // 128):
                        ft = p1sb.tile([128, 4, 128], f32, tag="ft")
                        nc.sync.dma_start(
                            out=ft[:],
                            in_=featT[:, t * 128:(t + 1) * 128].rearrange(
                                "(c p) j -> p c j", p=128))
                        fr = p1sb.tile([128, 4, 128], f32r, tag="fr")
                        nc.vector.tensor_copy(out=fr[:], in_=ft[:])
                        cjt = p1sb.tile([128, 1], f32, tag="cj")
                        nc.sync.dma_start(out=cjt[:],
                                          in_=cjd[t * 128:(t + 1) * 128, :])
                        pa = p1ps.tile([128, 320], f32, tag="pa")
                        pb = p1ps.tile([128, 320], f32, tag="pb")
                        for c in range(4):
                            nc.tensor.matmul(out=pa[:], lhsT=fr[:, c, :],
                                             rhs=w_sb[nm][:, c, 0:320],
                                             start=(c == 0), stop=(c == 3))
                            nc.tensor.matmul(out=pb[:], lhsT=fr[:, c, :],
                                             rhs=w_sb[nm][:, c, 320:640],
                                             start=(c == 0), stop=(c == 3))
                        ob = p1sb.tile([128, MSG], bf16, tag="ob")
                        nc.vector.tensor_scalar(
                            out=ob[:, 0:320], in0=pa[:], scalar1=cjt[:],
                            scalar2=None, op0=mybir.AluOpType.mult)
                        nc.vector.tensor_scalar(
                            out=ob[:, 320:640], in0=pb[:], scalar1=cjt[:],
                            scalar2=None, op0=mybir.AluOpType.mult)
                        nc.sync.dma_start(
                            out=agin[t * 128:(t + 1) * 128, :], in_=ob[:])

            # ---------- allgather ----------
            nc.gpsimd.collective_compute(
                "AllGather", mybir.AluOpType.bypass,
                replica_groups=[list(range(NCORES))],
                ins=[ag_in_u.opt()], outs=[xu_all.opt()])
            nc.gpsimd.collective_compute(
                "AllGather", mybir.AluOpType.bypass,
                replica_groups=[list(range(NCORES))],
                ins=[ag_in_i.opt()], outs=[xi_all.opt()])
            nc.gpsimd.dma_start(out=dbg[:], in_=xu_all[12416:12672, :])

            # ---------- phase 2 + 3 per direction ----------
            for dirname, meta, table, idx_d, fc_nm, cid, outd, tot_rows in (
                ("ui", ui, xu_all, idx_ui, "i", ci_i, out_i, U_ROWS),
                ("iu", iu, xi_all, idx_iu, "u", ci_u, out_u, I_ROWS),
            ):
                if lim == "p1" or (lim == "ui1" and dirname != "ui"):
                    continue
                n_tiles, n_bands = meta["n_tiles"], meta["n_bands"]
                chunk_pos = meta["chunk_pos"]
                max_cols = meta["max_slab_cols"]
                with (
                    tc.tile_pool(name=f"slab_{dirname}", bufs=3) as slpool,
                    tc.tile_pool(name=f"gidx_{dirname}", bufs=3) as gxpool,
                    tc.tile_pool(name=f"h_{dirname}", bufs=BAND + 4) as hpool,
                    tc.tile_pool(name=f"o_{dirname}", bufs=3) as opool,
                    tc.tile_pool(name=f"s_{dirname}", bufs=4) as spool,
                    tc.tile_pool(name=f"ps_{dirname}", bufs=3,
                                 space="PSUM") as pspool,
                    tc.tile_pool(name=f"fps_{dirname}", bufs=2,
                                 space="PSUM") as fpspool,
                ):
                    for b in range(n_bands if lim != "ui1" else 1):
                        t0, t1 = b * BAND, min((b + 1) * BAND, n_tiles)
                        hts = {}
                        for t in range(t0, t1):
                            hts[t] = hpool.tile([128, R, T_DST], f32r,
                                                name=f"hT_{dirname}_{t}",
                                                tag="hT")
                        for r in range(R):
                            sl = slpool.tile([128, max_cols, 128], bf16,
                                             tag="slab")
                            for (w, nch, soff, coff) in meta["calls"][b][r]:
                                for c0 in range(0, nch, 8):
                                    cn = min(8, nch - c0)
                                    so = soff + c0 * 128
                                    it = gxpool.tile([128, cn * 8], i16,
                                                     tag="gi")
                                    nc.sync.dma_start(
                                        out=it[:],
                                        in_=idx_d[:, so // 16:
                                                  (so + cn * 128) // 16])
                                    nc.gpsimd.dma_gather(
                                        out_ap=sl[:, coff + c0:coff + c0 + cn, :],
                                        in_ap=table[w * WIN:(w + 1) * WIN,
                                                    r * K:(r + 1) * K],
                                        idxs_ap=it[:],
                                        num_idxs=cn * 128,
                                        num_idxs_reg=cn * 128,
                                        elem_size=K,
                                        elem_step=MSG,
                                    )
                            for t in range(t0, t1):
                                chunks = chunk_pos[r][t]
                                if not chunks:
                                    nc.vector.memset(hts[t][:, r, :], 0.0)
                                    continue
                                ph = pspool.tile([128, T_DST], f32, tag="ph")
                                for j, (col, gcol) in enumerate(chunks):
                                    S = spool.tile([128, T_DST], bf16, tag="S")
                                    nc.vector.tensor_scalar(
                                        out=S[:], in0=iota_sb[:],
                                        scalar1=dcol_sb[dirname][:, gcol:gcol + 1],
                                        scalar2=None,
                                        op0=mybir.AluOpType.is_equal)
                                    nc.tensor.matmul(
                                        out=ph[:], lhsT=sl[:, col, :],
                                        rhs=S[:], start=(j == 0),
                                        stop=(j == len(chunks) - 1))
                                nc.vector.tensor_copy(out=hts[t][:, r, :],
                                                      in_=ph[:])
                        for t in range(t0, t1):
                            pf = fpspool.tile([T_DST, OUT], f32, tag="pf")
                            for r in range(R):
                                nc.tensor.matmul(out=pf[:], lhsT=hts[t][:, r, :],
                                                 rhs=fc_sb[fc_nm][:, r, :],
                                                 start=(r == 0),
                                                 stop=(r == R - 1))
                            cit = opool.tile([T_DST, 1], f32, tag="ci")
                            nc.sync.dma_start(
                                out=cit[:],
                                in_=cid[t * T_DST:(t + 1) * T_DST, :])
                            of = opool.tile([T_DST, OUT], f32, tag="of")
                            nc.vector.tensor_scalar(
                                out=of[:], in0=pf[:], scalar1=cit[:],
                                scalar2=None, op0=mybir.AluOpType.mult)
                            nc.vector.tensor_tensor(
                                out=of[:], in0=of[:],
                                in1=bias_sb[fc_nm][0:T_DST, :],
                                op=mybir.AluOpType.add)
                            nc.sync.dma_start(
                                out=outd[t * T_DST:(t + 1) * T_DST, :],
                                in_=of[:])

    if verbose:
        print(f"instructions: {len(nc.inst_map)}")
    nc.compile()
    return nc


def _prepare(inputs):
    ufeat = np.asarray(inputs["ufeat"], np.float32)
    ifeat = np.asarray(inputs["ifeat"], np.float32)
    W_u = np.asarray(inputs["W_u"], np.float32)
    W_i = np.asarray(inputs["W_i"], np.float32)
    user_ci = np.asarray(inputs["user_ci"], np.float32)
    user_cj = np.asarray(inputs["user_cj"], np.float32)
    movie_ci = np.asarray(inputs["movie_ci"], np.float32)
    movie_cj = np.asarray(inputs["movie_cj"], np.float32)
    edge_u = np.asarray(inputs["edge_u"], np.int64)
    edge_i = np.asarray(inputs["edge_i"], np.int64)
    ufc_w = np.asarray(inputs["ufc_w"], np.float32)
    ufc_b = np.asarray(inputs["ufc_b"], np.float32)
    ifc_w = np.asarray(inputs["ifc_w"], np.float32)
    ifc_b = np.asarray(inputs["ifc_b"], np.float32)

    ui = _prep_direction(edge_u, edge_i, I_SH, I_TILES, U_SH, U_PAD, 4)
    iu = _prep_direction(edge_i, edge_u, U_SH, U_TILES, I_SH, I_PAD, 2)

    featT_u = _shard_pad_T(ufeat, U_SH, U_PAD)
    featT_i = _shard_pad_T(ifeat, I_SH, I_PAD)
    cj_u = _shard_pad_vec(user_cj, U_SH, U_PAD)
    cj_i = _shard_pad_vec(movie_cj, I_SH, I_PAD)
    ci_u = _shard_pad_vec(user_ci, U_SH, U_OPAD)
    ci_i = _shard_pad_vec(movie_ci, I_SH, I_OPAD)

    wcat_u = np.ascontiguousarray(W_u.transpose(1, 0, 2).reshape(DIN, MSG))
    wcat_i = np.ascontiguousarray(W_i.transpose(1, 0, 2).reshape(DIN, MSG))
    fcw_u = np.ascontiguousarray(ufc_w.reshape(R, K, OUT))
    fcw_i = np.ascontiguousarray(ifc_w.reshape(R, K, OUT))
    bias_u = np.tile(ufc_b[None, :], (128, 1)).astype(np.float32)
    bias_i = np.tile(ifc_b[None, :], (128, 1)).astype(np.float32)
    iota = np.tile(np.arange(T_DST, dtype=np.float32)[None, :],
                   (128, 1)).astype(BF16)

    in_maps = []
    for c in range(NCORES):
        in_maps.append({
            "featT_u": featT_u[c], "featT_i": featT_i[c],
            "wcat_u": wcat_u, "wcat_i": wcat_i,
            "cj_u": cj_u[c], "cj_i": cj_i[c],
            "ci_u": ci_u[c], "ci_i": ci_i[c],
            "fcw_u": fcw_u, "fcw_i": fcw_i,
            "bias_u": bias_u, "bias_i": bias_i,
            "iota_t": iota,
            "idx_ui": ui["idx16"][c], "idx_iu": iu["idx16"][c],
            "dcol_ui": ui["dcols"][c], "dcol_iu": iu["dcols"][c],
        })
    return ui, iu, in_maps


def kernel(**inputs):
    import os
    ui, iu, in_maps = _prepare(inputs)
    nc = _build_program(ui, iu, verbose=True)
    trace = bool(int(os.environ.get("KTRACE", "0")))
    res = run_bass_kernel_spmd(nc, in_maps, core_ids=list(range(NCORES)),
                               trace=trace)
    _CACHE["last_result"] = res
    u_out = np.concatenate(
        [res.results[c]["out_u"][:U_SH] for c in range(NCORES)], axis=0)
    i_out = np.concatenate(
        [res.results[c]["out_i"][:I_SH] for c in range(NCORES)], axis=0)
    return u_out, i_out


# revision 16
# speedup vs baseline: 1.4263x; 1.4263x over previous
"""GCMC layer kernel for 8 Trainium2 NeuronCores (Bass/Tile).

Strategy (dest-sharded message passing, one shared SPMD NEFF):
  Phase 1: each core transforms its node shard: xu[row, 5*128] = featT.T @ Wcat
           (fp32r matmuls), scaled by cj, cast to bf16 -> local DRAM table.
  AllGather: full bf16 message tables on every core.
  Phase 2: per (dest-tile, rating): dma_gather per-edge source messages
           (int16 indices rebased per 25088-row window), build one-hot S from
           dest columns on DVE, accumulate h_T[k, dest] = M.T @ S in PSUM.
  Phase 3: FC h_T.T @ fc_w accumulated over ratings (fp32r), scale by ci,
           add bias, write output shards; host concatenates.

Host precomputes: transposed features, per-core edge orderings sorted by
(band, rating, window, dest-tile), a max-over-cores common chunk schedule so
all 8 cores share one instruction stream, int16 gather indices, and per-edge
dest-column tables in processing order.
"""
import os

import numpy as np
import ml_dtypes

from concourse import bacc, mybir, tile
from concourse.bass_utils import run_bass_kernel_spmd

# Problem constants (from spec)
R = 5
NU = 100000
NI = 50000
DIN = 512
K = 128
MSG = R * K            # 640
OUT = 256
E = 200000
NCORES = 8

U_SH = NU // NCORES    # 12500 users per core
I_SH = NI // NCORES    # 6250 movies per core
U_PAD = 12544          # table rows per user shard (98 * 128)
I_PAD = 6272           # table rows per movie shard (49 * 128)
U_ROWS = U_PAD * NCORES   # 100352 (4 windows of 25088)
I_ROWS = I_PAD * NCORES   # 50176  (2 windows of 25088)
WIN = 25088               # equal-size index windows (<= 32768 for int16)

T_DST = 96             # dest-tile span for phase 2/3
U_TILES = -(-U_PAD // T_DST)   # 131 user dest tiles
I_TILES = -(-I_PAD // T_DST)   # 66 movie dest tiles
U_OPAD = U_TILES * T_DST       # 12576 output rows per core
I_OPAD = I_TILES * T_DST       # 6336
BAND = 16              # dest tiles per gather band
OBAND = 8              # dest tiles per output-staging DMA
GMAX = 8               # max chunks per dma_gather call (ucode ~1024-idx cap)

BF16 = ml_dtypes.bfloat16

_CACHE = {}


def _shard_pad_T(x, sh, pad):
    """[N, D] -> per-core transposed padded [D, pad] fp32 list."""
    out = []
    for c in range(NCORES):
        blk = x[c * sh:(c + 1) * sh]
        t = np.zeros((x.shape[1], pad), np.float32)
        t[:, :sh] = blk.T
        out.append(np.ascontiguousarray(t))
    return out


def _shard_vec_T(v, sh, pad, p):
    """per-core [p, pad//p] fp32 column-per-tile layout of a padded vector."""
    out = []
    for c in range(NCORES):
        t = np.zeros(pad, np.float32)
        t[:sh] = v[c * sh:(c + 1) * sh, 0]
        out.append(np.ascontiguousarray(t.reshape(-1, p).T))
    return out


def _prep_direction(edge_src, edge_dst, dst_sh, n_tiles, src_sh, src_pad, n_win):
    """Shared chunk schedule + per-core gather-index / dest-column arrays.

    Slot layout: bands outer; within band: r, then w, then t. All 8 cores pad
    each (r, t, w) cell to the max-over-cores chunk count so the instruction
    stream is identical. dcol arrays are emitted in processing order
    (r, t, chunk) so per-(t, r) chunk columns are consecutive.
    """
    core_of = edge_dst // dst_sh
    dloc = edge_dst - core_of * dst_sh
    t_of = dloc // T_DST
    dcol_of = dloc - t_of * T_DST
    srow = (edge_src // src_sh) * src_pad + (edge_src % src_sh)
    w_of = srow // WIN

    n_bands = -(-n_tiles // BAND)
    counts = np.zeros((NCORES, R, n_tiles, n_win), np.int64)
    for r in range(R):
        for c in range(NCORES):
            m = core_of[r] == c
            key = t_of[r][m] * n_win + w_of[r][m]
            counts[c, r] += np.bincount(key, minlength=n_tiles * n_win).reshape(
                n_tiles, n_win)
    cell_chunks = -(-counts.max(axis=0) // 128)       # [R, n_tiles, n_win]

    calls = [[[] for _ in range(R)] for _ in range(n_bands)]
    slab_cols = [[0] * R for _ in range(n_bands)]
    # per (r, t): list of (col_in_slab, slot_chunk_index)
    raw_chunks = [[[] for _ in range(n_tiles)] for _ in range(R)]
    cell_slot = np.zeros((R, n_tiles, n_win), np.int64)
    slot_off = 0
    for b in range(n_bands):
        t0, t1 = b * BAND, min((b + 1) * BAND, n_tiles)
        for r in range(R):
            col = 0
            for w in range(n_win):
                nch = int(cell_chunks[r, t0:t1, w].sum())
                if nch == 0:
                    continue
                calls[b][r].append((w, nch, slot_off, col))
                cc = col
                for t in range(t0, t1):
                    for j in range(int(cell_chunks[r, t, w])):
                        if j == 0:
                            cell_slot[r, t, w] = slot_off + (cc - col) * 128
                        raw_chunks[r][t].append((cc, slot_off // 128 + cc - col))
                        cc += 1
                slot_off += nch * 128
                col = cc
            slab_cols[b][r] = col
    total_slots = slot_off
    total_chunks = slot_off // 128

    # processing order: r outer, t inner; assign pcol
    chunk_pos = [[[] for _ in range(n_tiles)] for _ in range(R)]
    sci_order = []
    pcol = 0
    maxch = 0
    for r in range(R):
        for t in range(n_tiles):
            for (col, sci) in raw_chunks[r][t]:
                chunk_pos[r][t].append((col, pcol))
                sci_order.append(sci)
                pcol += 1
            maxch = max(maxch, len(raw_chunks[r][t]))
    sci_order = np.asarray(sci_order, np.int64)
    assert pcol == total_chunks

    idx16, dcols = [], []
    for c in range(NCORES):
        idx_arr = np.zeros(total_slots, np.int16)
        dcol_arr = np.full(total_slots, -1, np.int16)
        for r in range(R):
            m = core_of[r] == c
            tt, ww = t_of[r][m], w_of[r][m]
            rows, dc = srow[r][m], dcol_of[r][m]
            order = np.lexsort((ww, tt))
            tt, ww, rows, dc = tt[order], ww[order], rows[order], dc[order]
            key = tt * n_win + ww
            uniq, first = np.unique(key, return_index=True)
            cnts = np.diff(np.append(first, len(key)))
            rank = np.arange(len(key)) - np.repeat(first, cnts)
            pos = cell_slot[r].reshape(-1)[key] + rank
            idx_arr[pos] = (rows - ww * WIN).astype(np.int16)
            dcol_arr[pos] = dc.astype(np.int16)
        iw = np.tile(idx_arr.reshape(-1, 16).T, (8, 1)).copy()
        # dcol in processing order: [128, total_chunks]
        dw = np.ascontiguousarray(dcol_arr.reshape(-1, 128)[sci_order].T)
        idx16.append(iw)
        dcols.append(dw)

    return dict(calls=calls, slab_cols=slab_cols, chunk_pos=chunk_pos,
                idx16=idx16, dcols=dcols, total_slots=total_slots,
                total_chunks=total_chunks, n_bands=n_bands, n_win=n_win,
                n_tiles=n_tiles, maxch=maxch,
                max_slab_cols=max(max(sc) for sc in slab_cols))


def _build_program(ui, iu, verbose=False):
    lim = os.environ.get("PHASE_LIMIT", "all")
    nc = bacc.Bacc("TRN2", target_bir_lowering=False, debug=False,
                   num_devices=NCORES)
    f32, f32r, bf16, i16 = (mybir.dt.float32, mybir.dt.float32r,
                            mybir.dt.bfloat16, mybir.dt.int16)
    MAXCH = max(ui["maxch"], iu["maxch"])

    def inp(name, shape, dt):
        return nc.dram_tensor(name, shape, dt, kind="ExternalInput").ap()

    featT_u = inp("featT_u", [DIN, U_PAD], f32r)
    featT_i = inp("featT_i", [DIN, I_PAD], f32r)
    wcat_u = inp("wcat_u", [DIN, MSG], f32r)
    wcat_i = inp("wcat_i", [DIN, MSG], f32r)
    cj_u = inp("cj_u", [128, U_PAD // 128], f32)
    cj_i = inp("cj_i", [128, I_PAD // 128], f32)
    ci_u = inp("ci_u", [T_DST, U_TILES], f32)
    ci_i = inp("ci_i", [T_DST, I_TILES], f32)
    fcw_u = inp("fcw_u", [R, K, OUT], f32r)
    fcw_i = inp("fcw_i", [R, K, OUT], f32r)
    bias_u = inp("bias_u", [128, OUT], f32)
    bias_i = inp("bias_i", [128, OUT], f32)
    iota_t = inp("iota_t", [128, MAXCH, T_DST], f32)
    idx_ui = inp("idx_ui", [128, ui["total_slots"] // 16], i16)
    idx_iu = inp("idx_iu", [128, iu["total_slots"] // 16], i16)
    dcol_ui = inp("dcol_ui", [128, ui["total_chunks"]], i16)
    dcol_iu = inp("dcol_iu", [128, iu["total_chunks"]], i16)

    out_u = nc.dram_tensor("out_u", [U_OPAD, OUT], f32, kind="ExternalOutput").ap()
    out_i = nc.dram_tensor("out_i", [I_OPAD, OUT], f32, kind="ExternalOutput").ap()
    dbg = nc.dram_tensor("dbg", [256, MSG], bf16, kind="ExternalOutput").ap()

    ag_in_u = nc.dram_tensor("ag_in_u", [U_PAD, MSG], bf16).ap()
    ag_in_i = nc.dram_tensor("ag_in_i", [I_PAD, MSG], bf16).ap()
    xu_all = nc.dram_tensor("xu_all", [U_ROWS, MSG], bf16, addr_space="Shared").ap()
    xi_all = nc.dram_tensor("xi_all", [I_ROWS, MSG], bf16, addr_space="Shared").ap()

    with tile.TileContext(nc) as tc:
        with tc.tile_pool(name="const", bufs=1) as cpool:
            # ---------- constants ----------
            w_sb, fc_sb, bias_sb, dcol_sb, cj_sb, ci_sb = {}, {}, {}, {}, {}, {}
            with tc.tile_pool(name="stage", bufs=2) as stg:
                for nm, dd, tot in (("ui", dcol_ui, ui["total_chunks"]),
                                    ("iu", dcol_iu, iu["total_chunks"])):
                    dt_ = stg.tile([128, tot], i16, tag="dst")
                    nc.sync.dma_start(out=dt_[:], in_=dd[:])
                    db = cpool.tile([128, tot], f32, tag=f"dcb_{nm}")
                    nc.vector.tensor_copy(out=db[:], in_=dt_[:])
                    dcol_sb[nm] = db
            for nm, wd in (("u", wcat_u), ("i", wcat_i)):
                wt = cpool.tile([128, 4, MSG], f32r, tag=f"wr_{nm}")
                nc.sync.dma_start(out=wt[:], in_=wd.rearrange(
                    "(c p) m -> p c m", p=128))
                w_sb[nm] = wt
            for nm, fd in (("u", fcw_u), ("i", fcw_i)):
                ft = cpool.tile([128, R, OUT], f32r, tag=f"fcr_{nm}")
                nc.sync.dma_start(out=ft[:], in_=fd.rearrange("r p o -> p r o"))
                fc_sb[nm] = ft
            for nm, bd in (("u", bias_u), ("i", bias_i)):
                bt = cpool.tile([128, OUT], f32, tag=f"b_{nm}")
                nc.sync.dma_start(out=bt[:], in_=bd[:])
                bias_sb[nm] = bt
            for nm, cd, ncol in (("u", cj_u, U_PAD // 128),
                                 ("i", cj_i, I_PAD // 128)):
                ct = cpool.tile([128, ncol], f32, tag=f"cj_{nm}")
                nc.sync.dma_start(out=ct[:], in_=cd[:])
                cj_sb[nm] = ct
            for nm, cd, ncol in (("u", ci_u, U_TILES), ("i", ci_i, I_TILES)):
                ct = cpool.tile([T_DST, ncol], f32, tag=f"ci_{nm}")
                nc.sync.dma_start(out=ct[:], in_=cd[:])
                ci_sb[nm] = ct
            iota_sb = cpool.tile([128, MAXCH, T_DST], f32)
            nc.sync.dma_start(out=iota_sb[:], in_=iota_t[:])

            # ---------- phase 1: transform own shards ----------
            with (
                tc.tile_pool(name="p1sb", bufs=3) as p1sb,
                tc.tile_pool(name="p1ps", bufs=3, space="PSUM") as p1ps,
            ):
                for nm, featT, agin, npad in (
                    ("u", featT_u, ag_in_u, U_PAD),
                    ("i", featT_i, ag_in_i, I_PAD),
                ):
                    for t in range(npad // 128):
                        fr = p1sb.tile([128, 4, 128], f32r, tag="fr")
                        nc.sync.dma_start(
                            out=fr[:],
                            in_=featT[:, t * 128:(t + 1) * 128].rearrange(
                                "(c p) j -> p c j", p=128))
                        pa = p1ps.tile([128, 320], f32, tag="pa")
                        pb = p1ps.tile([128, 320], f32, tag="pb")
                        for c in range(4):
                            nc.tensor.matmul(out=pa[:], lhsT=fr[:, c, :],
                                             rhs=w_sb[nm][:, c, 0:320],
                                             start=(c == 0), stop=(c == 3))
                            nc.tensor.matmul(out=pb[:], lhsT=fr[:, c, :],
                                             rhs=w_sb[nm][:, c, 320:640],
                                             start=(c == 0), stop=(c == 3))
                        ob = p1sb.tile([128, MSG], bf16, tag="ob")
                        nc.vector.tensor_scalar(
                            out=ob[:, 0:320], in0=pa[:],
                            scalar1=cj_sb[nm][:, t:t + 1],
                            scalar2=None, op0=mybir.AluOpType.mult)
                        nc.vector.tensor_scalar(
                            out=ob[:, 320:640], in0=pb[:],
                            scalar1=cj_sb[nm][:, t:t + 1],
                            scalar2=None, op0=mybir.AluOpType.mult)
                        nc.sync.dma_start(
                            out=agin[t * 128:(t + 1) * 128, :], in_=ob[:])

            # ---------- allgather ----------
            if lim == "nocc":
                nc.gpsimd.dma_start(out=xu_all[0:U_PAD, :], in_=ag_in_u[:])
                nc.gpsimd.dma_start(out=xi_all[0:I_PAD, :], in_=ag_in_i[:])
            else:
                nc.gpsimd.collective_compute(
                    "AllGather", mybir.AluOpType.bypass,
                    replica_groups=[list(range(NCORES))],
                    ins=[ag_in_u.opt()], outs=[xu_all.opt()])
                nc.gpsimd.collective_compute(
                    "AllGather", mybir.AluOpType.bypass,
                    replica_groups=[list(range(NCORES))],
                    ins=[ag_in_i.opt()], outs=[xi_all.opt()])
            nc.gpsimd.dma_start(out=dbg[:], in_=xu_all[12416:12672, :])

            # ---------- phase 2 + 3 per direction ----------
            for dirname, meta, table, idx_d, fc_nm, outd, tot_rows in (
                ("ui", ui, xu_all, idx_ui, "i", out_i, U_ROWS),
                ("iu", iu, xi_all, idx_iu, "u", out_u, I_ROWS),
            ):
                if lim == "p1" or (lim == "ui1" and dirname != "ui"):
                    continue
                n_tiles, n_bands = meta["n_tiles"], meta["n_bands"]
                chunk_pos = meta["chunk_pos"]
                max_cols = meta["max_slab_cols"]
                with (
                    tc.tile_pool(name=f"idx_{dirname}", bufs=1) as ixpool,
                    tc.tile_pool(name=f"slab_{dirname}", bufs=2) as slpool,
                    tc.tile_pool(name=f"h_{dirname}", bufs=BAND + 2) as hpool,
                    tc.tile_pool(name=f"o_{dirname}", bufs=2) as opool,
                    tc.tile_pool(name=f"s_{dirname}", bufs=4) as spool,
                    tc.tile_pool(name=f"ps_{dirname}", bufs=3,
                                 space="PSUM") as pspool,
                    tc.tile_pool(name=f"fps_{dirname}", bufs=2,
                                 space="PSUM") as fpspool,
                ):
                    idx_sb = ixpool.tile([128, meta["total_slots"] // 16], i16,
                                         tag="ix")
                    nc.sync.dma_start(out=idx_sb[:], in_=idx_d[:])
                    for b in range(n_bands if lim != "ui1" else 1):
                        t0, t1 = b * BAND, min((b + 1) * BAND, n_tiles)
                        hts = {}
                        for t in range(t0, t1):
                            hts[t] = hpool.tile([128, R, T_DST], f32r,
                                                name=f"hT_{dirname}_{t}",
                                                tag="hT")
                        ost = {}
                        for ob0 in range(t0, t1, OBAND):
                            ost[ob0] = opool.tile(
                                [T_DST, min(OBAND, n_tiles - ob0), OUT], f32,
                                name=f"ost_{dirname}_{ob0}", tag="ost")
                        for r in range(R):
                            sl = slpool.tile([128, max_cols, 128], bf16,
                                             tag="slab")
                            for (w, nch, soff, coff) in meta["calls"][b][r]:
                                for c0 in range(0, nch, GMAX):
                                    cn = min(GMAX, nch - c0)
                                    so = soff + c0 * 128
                                    nc.gpsimd.dma_gather(
                                        out_ap=sl[:, coff + c0:coff + c0 + cn, :],
                                        in_ap=table[w * WIN:(w + 1) * WIN,
                                                    r * K:(r + 1) * K],
                                        idxs_ap=idx_sb[:, so // 16:
                                                       (so + cn * 128) // 16],
                                        num_idxs=cn * 128,
                                        num_idxs_reg=cn * 128,
                                        elem_size=K,
                                        elem_step=MSG,
                                    )
                            for t in range(t0, t1):
                                chunks = chunk_pos[r][t]
                                if not chunks:
                                    nc.vector.memset(hts[t][:, r, :], 0.0)
                                    continue
                                nch = len(chunks)
                                p0 = chunks[0][1]
                                S = spool.tile([128, MAXCH, T_DST], bf16,
                                               tag="S")
                                nc.vector.tensor_tensor(
                                    out=S[:, 0:nch, :],
                                    in0=dcol_sb[dirname][:, p0:p0 + nch]
                                    .to_broadcast([128, nch, T_DST]),
                                    in1=iota_sb[:, 0:nch, :],
                                    op=mybir.AluOpType.is_equal)
                                ph = pspool.tile([128, T_DST], f32, tag="ph")
                                for j, (col, pc) in enumerate(chunks):
                                    nc.tensor.matmul(
                                        out=ph[:], lhsT=sl[:, col, :],
                                        rhs=S[:, j, :], start=(j == 0),
                                        stop=(j == nch - 1))
                                nc.vector.tensor_copy(out=hts[t][:, r, :],
                                                      in_=ph[:])
                        for t in range(t0, t1):
                            pf = fpspool.tile([T_DST, OUT], f32, tag="pf")
                            for r in range(R):
                                nc.tensor.matmul(out=pf[:], lhsT=hts[t][:, r, :],
                                                 rhs=fc_sb[fc_nm][:, r, :],
                                                 start=(r == 0),
                                                 stop=(r == R - 1))
                            ob0 = t0 + ((t - t0) // OBAND) * OBAND
                            stile = ost[ob0]
                            nc.vector.tensor_scalar(
                                out=stile[:, t - ob0, :], in0=pf[:],
                                scalar1=ci_sb[fc_nm][:, t:t + 1],
                                scalar2=None, op0=mybir.AluOpType.mult)
                            nc.vector.tensor_tensor(
                                out=stile[:, t - ob0, :],
                                in0=stile[:, t - ob0, :],
                                in1=bias_sb[fc_nm][0:T_DST, :],
                                op=mybir.AluOpType.add)
                        for ob0, stile in ost.items():
                            nb = stile.shape[1]
                            nc.sync.dma_start(
                                out=outd[ob0 * T_DST:(ob0 + nb) * T_DST, :]
                                .rearrange("(t p) o -> p t o", p=T_DST),
                                in_=stile[:])

    if verbose:
        print(f"instructions: {len(nc.inst_map)}")
    nc.compile()
    return nc


def _prepare(inputs):
    ufeat = np.asarray(inputs["ufeat"], np.float32)
    ifeat = np.asarray(inputs["ifeat"], np.float32)
    W_u = np.asarray(inputs["W_u"], np.float32)
    W_i = np.asarray(inputs["W_i"], np.float32)
    user_ci = np.asarray(inputs["user_ci"], np.float32)
    user_cj = np.asarray(inputs["user_cj"], np.float32)
    movie_ci = np.asarray(inputs["movie_ci"], np.float32)
    movie_cj = np.asarray(inputs["movie_cj"], np.float32)
    edge_u = np.asarray(inputs["edge_u"], np.int64)
    edge_i = np.asarray(inputs["edge_i"], np.int64)
    ufc_w = np.asarray(inputs["ufc_w"], np.float32)
    ufc_b = np.asarray(inputs["ufc_b"], np.float32)
    ifc_w = np.asarray(inputs["ifc_w"], np.float32)
    ifc_b = np.asarray(inputs["ifc_b"], np.float32)

    ui = _prep_direction(edge_u, edge_i, I_SH, I_TILES, U_SH, U_PAD, 4)
    iu = _prep_direction(edge_i, edge_u, U_SH, U_TILES, I_SH, I_PAD, 2)
    maxch = max(ui["maxch"], iu["maxch"])

    featT_u = _shard_pad_T(ufeat, U_SH, U_PAD)
    featT_i = _shard_pad_T(ifeat, I_SH, I_PAD)
    cj_u = _shard_vec_T(user_cj, U_SH, U_PAD, 128)
    cj_i = _shard_vec_T(movie_cj, I_SH, I_PAD, 128)
    ci_u = _shard_vec_T(user_ci, U_SH, U_OPAD, T_DST)
    ci_i = _shard_vec_T(movie_ci, I_SH, I_OPAD, T_DST)

    wcat_u = np.ascontiguousarray(W_u.transpose(1, 0, 2).reshape(DIN, MSG))
    wcat_i = np.ascontiguousarray(W_i.transpose(1, 0, 2).reshape(DIN, MSG))
    fcw_u = np.ascontiguousarray(ufc_w.reshape(R, K, OUT))
    fcw_i = np.ascontiguousarray(ifc_w.reshape(R, K, OUT))
    bias_u = np.tile(ufc_b[None, :], (128, 1)).astype(np.float32)
    bias_i = np.tile(ifc_b[None, :], (128, 1)).astype(np.float32)
    iota = np.tile(np.arange(T_DST, dtype=np.float32)[None, None, :],
                   (128, maxch, 1)).astype(np.float32)

    in_maps = []
    for c in range(NCORES):
        in_maps.append({
            "featT_u": featT_u[c], "featT_i": featT_i[c],
            "wcat_u": wcat_u, "wcat_i": wcat_i,
            "cj_u": cj_u[c], "cj_i": cj_i[c],
            "ci_u": ci_u[c], "ci_i": ci_i[c],
            "fcw_u": fcw_u, "fcw_i": fcw_i,
            "bias_u": bias_u, "bias_i": bias_i,
            "iota_t": iota,
            "idx_ui": ui["idx16"][c], "idx_iu": iu["idx16"][c],
            "dcol_ui": ui["dcols"][c], "dcol_iu": iu["dcols"][c],
        })
    return ui, iu, in_maps


def kernel(**inputs):
    ui, iu, in_maps = _prepare(inputs)
    nc = _build_program(ui, iu, verbose=True)
    trace = bool(int(os.environ.get("KTRACE", "0")))
    res = run_bass_kernel_spmd(nc, in_maps, core_ids=list(range(NCORES)),
                               trace=trace)
    _CACHE["last_result"] = res
    u_out = np.concatenate(
        [res.results[c]["out_u"][:U_SH] for c in range(NCORES)], axis=0)
    i_out = np.concatenate(
        [res.results[c]["out_i"][:I_SH] for c in range(NCORES)], axis=0)
    return u_out, i_out
